# revision 16
# baseline (speedup 1.0000x reference)
"""DropoutDPP eval-path kernel for 8 Trainium2 NeuronCores.

The reference accumulates Bernoulli keep-masks (fixed RNG key 42, independent
of x) over the hidden dim until >=40% of neurons are nonzero, then computes
x * sum_mask / i.  The mask state is a deterministic constant, so it is
computed host-side (same jax threefry bits as the reference) and folded into a
single [hidden] scale vector.  The heavy, memory-bound part — scaling the
[4, 4096, 4096] tensor along its last dim — runs on 8 cores, data-parallel
over rows.

The on-device kernel is hand-scheduled raw Bass (this toolchain's TT struct
encodes a single sync wait, which rules out Tile's multi-wait scheduling):
SP issues loads, DVE multiplies in place, ACT issues stores; three 4MB SBUF
slots rotate.
"""

import numpy as np

_B, _S, _H = 4, 4096, 4096
_N_CORES = 8
_ROWS = _B * _S                       # 16384
_ROWS_PER_CORE = _ROWS // _N_CORES    # 2048
_P = 0.9
_MAX_N = 100
_MAX_FRAC = 0.4

_J = 1                                # 128-row blocks per SBUF tile (2MB tiles)
_ROWS_PER_TILE = 128 * _J
_N_TILES = _ROWS_PER_CORE // _ROWS_PER_TILE
_N_SLOTS = 8

_cache: dict = {}


def _compute_scale() -> np.ndarray:
    """Replicate reference._accumulate_masks exactly (threefry is
    backend/platform deterministic), returning sum_mask / i as float32."""
    if "scale" in _cache:
        return _cache["scale"]
    import jax
    import jax.numpy as jnp

    cpu = jax.devices("cpu")[0]
    with jax.default_device(cpu):
        key = jax.random.key(42)
        key, k0 = jax.random.split(key)
        sum_mask = (jax.random.uniform(k0, (_H,)) >= _P).astype(jnp.float32)
        i = 1
        while i < _MAX_N and float(
            jnp.mean((sum_mask != 0).astype(jnp.float32))
        ) < _MAX_FRAC:
            key, k = jax.random.split(key)
            sum_mask = sum_mask + (jax.random.uniform(k, (_H,)) >= _P).astype(
                jnp.float32
            )
            i += 1
    scale = np.asarray(sum_mask, dtype=np.float32) / np.float32(i)
    _cache["scale"] = scale
    return scale


def _build_nc():
    if "nc" in _cache:
        return _cache["nc"]
    import concourse.bass as bass
    import concourse.mybir as mybir
    from contextlib import ExitStack

    nc = bass.Bass(trn_type="TRN2")
    x = nc.dram_tensor(
        "x", [_ROWS_PER_CORE, _H], mybir.dt.float32, kind="ExternalInput"
    )
    scale = nc.dram_tensor(
        "scale", [1, _H], mybir.dt.float32, kind="ExternalInput"
    )
    y = nc.dram_tensor(
        "y", [_ROWS_PER_CORE, _H], mybir.dt.float32, kind="ExternalOutput"
    )

    xv = x[:, :].rearrange("(n p) h -> n p h", p=128)
    yv = y[:, :].rearrange("(n p) h -> n p h", p=128)

    with ExitStack() as ctx:
        scale_in = ctx.enter_context(
            nc.sbuf_tensor("scale_in", [1, _H], mybir.dt.float32)
        )
        ones_sb = ctx.enter_context(
            nc.sbuf_tensor("ones_sb", [1, 128], mybir.dt.float32)
        )
        scale_sb = ctx.enter_context(
            nc.sbuf_tensor("scale_sb", [128, _H], mybir.dt.float32)
        )
        psum_bc = ctx.enter_context(
            nc.psum_tensor("psum_bc", [128, _H], mybir.dt.float32)
        )
        slots = [
            ctx.enter_context(
                nc.sbuf_tensor(f"slot{s}", [128, _H], mybir.dt.float32)
            )
            for s in range(_N_SLOTS)
        ]
        # One semaphore per slot: each slot has at most one outstanding DMA
        # at a time (load +16, store +16 → +32 per slot cycle), making the
        # wait thresholds exact.  A single shared DMA sem would race: the 16
        # SDMA engines increment independently per transfer, so "sem >=
        # 16*(i+1)" does not imply transfers 0..i all completed.
        slot_sems = [
            ctx.enter_context(nc.semaphore(f"slot_sem{s}"))
            for s in range(_N_SLOTS)
        ]
        mul_sem = ctx.enter_context(nc.semaphore("mul_sem"))
        sc_sem = ctx.enter_context(nc.semaphore("sc_sem"))
        ones_sem = ctx.enter_context(nc.semaphore("ones_sem"))
        pe_sem = ctx.enter_context(nc.semaphore("pe_sem"))
        bc_sem = ctx.enter_context(nc.semaphore("bc_sem"))
        block = ctx.enter_context(nc.Block())

        n_cycles = _N_TILES // _N_SLOTS

        @block.gpsimd
        def _(gpsimd):
            # 16KB scale row via SWDGE — off the load/store HWDGE rings
            gpsimd.memset(ones_sb[:, :], 1.0).then_inc(ones_sem, 1)
            gpsimd.dma_start(out=scale_in[:, :], in_=scale[:, :]).then_inc(
                sc_sem, 16
            )

        @block.tensor
        def _(tensor):
            # rank-1 broadcast: ones[128] x scale[h] fans the scale row out
            # to all 128 partitions, one 512-wide matmul per PSUM bank
            tensor.wait_ge(sc_sem, 16)
            tensor.wait_ge(ones_sem, 1)
            for c in range(_H // 512):
                nc.tensor.matmul(
                    psum_bc[:, c * 512 : (c + 1) * 512],
                    lhsT=ones_sb[:, :],
                    rhs=scale_in[:, c * 512 : (c + 1) * 512],
                    start=True,
                    stop=True,
                ).then_inc(pe_sem, 1)

        @block.sync
        def _(sync):
            for i in range(_N_TILES):
                s, cyc = i % _N_SLOTS, i // _N_SLOTS
                if cyc > 0:
                    sync.wait_ge(slot_sems[s], 32 * cyc)  # prev store landed
                sync.dma_start(out=slots[s][:, :], in_=xv[i]).then_inc(
                    slot_sems[s], 16
                )

        @block.vector
        def _(vector):
            vector.wait_ge(pe_sem, _H // 512)
            vector.tensor_copy(out=scale_sb[:, :], in_=psum_bc[:, :]).then_inc(
                bc_sem, 1
            )
            # self-wait: ensure the copy's writes drained before muls read them
            vector.wait_ge(bc_sem, 1)
            for i in range(_N_TILES):
                s, cyc = i % _N_SLOTS, i // _N_SLOTS
                vector.wait_ge(slot_sems[s], 32 * cyc + 16)  # this load landed
                t = slots[s]
                vector.tensor_mul(
                    out=t[:, :], in0=t[:, :], in1=scale_sb[:, :]
                ).then_inc(mul_sem, 1)

        @block.scalar
        def _(scalar):
            for i in range(_N_TILES):
                s = i % _N_SLOTS
                scalar.wait_ge(mul_sem, i + 1)
                scalar.dma_start(out=yv[i], in_=slots[s][:, :]).then_inc(
                    slot_sems[s], 16
                )
            # all output bytes landed before the program ends
            for s in range(_N_SLOTS):
                scalar.wait_ge(slot_sems[s], 32 * n_cycles)

    _cache["nc"] = nc
    return nc


def _run(x: np.ndarray, trace: bool = False, trace_cores=None):
    """Returns (full_output, BassKernelResults)."""
    from concourse.bass_utils import run_bass_kernel_spmd

    nc = _build_nc()
    scale_row = np.ascontiguousarray(_compute_scale().reshape(1, _H))
    xf = np.ascontiguousarray(x, dtype=np.float32).reshape(_ROWS, _H)
    in_maps = [
        {"x": xf[c * _ROWS_PER_CORE : (c + 1) * _ROWS_PER_CORE], "scale": scale_row}
        for c in range(_N_CORES)
    ]
    res = run_bass_kernel_spmd(
        nc,
        in_maps,
        core_ids=list(range(_N_CORES)),
        trace=trace,
        trace_cores=trace_cores,
    )
    out = np.concatenate([r["y"] for r in res.results], axis=0)
    return out.reshape(_B, _S, _H), res


def kernel(**inputs) -> np.ndarray:
    out, _ = _run(np.asarray(inputs["x"]))
    return out


# revision 19
# speedup vs baseline: 1.2262x; 1.2262x over previous
"""DropoutDPP eval-path kernel for 8 Trainium2 NeuronCores.

The reference accumulates Bernoulli keep-masks (fixed RNG key 42, independent
of x) over the hidden dim until >=40% of neurons are nonzero, then computes
x * sum_mask / i.  The mask state is a deterministic constant, so it is
computed host-side (same jax threefry bits as the reference) and folded into a
single [hidden] scale vector.  The heavy, memory-bound part — scaling the
[4, 4096, 4096] tensor along its last dim — runs on 8 cores, data-parallel
over rows.

The on-device kernel is hand-scheduled raw Bass (this toolchain's TT struct
encodes a single sync wait, which rules out Tile's multi-wait scheduling):
SP issues the scale load then x-tile loads, DVE multiplies in place, ACT
issues stores; eight 1MB SBUF slots rotate, one semaphore per slot so wait
thresholds are exact.
"""

import numpy as np

_B, _S, _H = 4, 4096, 4096
_N_CORES = 8
_ROWS = _B * _S                       # 16384
_ROWS_PER_CORE = _ROWS // _N_CORES    # 2048
_P = 0.9
_MAX_N = 100
_MAX_FRAC = 0.4

_COLS_PER_TILE = 2048                 # [128, 2048] f32 = 1MB tiles
_TILES_PER_ROWBLK = _H // _COLS_PER_TILE
_N_ROWBLKS = _ROWS_PER_CORE // 128    # 16
_N_TILES = _N_ROWBLKS * _TILES_PER_ROWBLK  # 32
_N_SLOTS = 8

_cache: dict = {}


def _compute_scale() -> np.ndarray:
    """Replicate reference._accumulate_masks exactly (threefry is
    backend/platform deterministic), returning sum_mask / i as float32."""
    if "scale" in _cache:
        return _cache["scale"]
    import jax
    import jax.numpy as jnp

    cpu = jax.devices("cpu")[0]
    with jax.default_device(cpu):
        key = jax.random.key(42)
        key, k0 = jax.random.split(key)
        sum_mask = (jax.random.uniform(k0, (_H,)) >= _P).astype(jnp.float32)
        i = 1
        while i < _MAX_N and float(
            jnp.mean((sum_mask != 0).astype(jnp.float32))
        ) < _MAX_FRAC:
            key, k = jax.random.split(key)
            sum_mask = sum_mask + (jax.random.uniform(k, (_H,)) >= _P).astype(
                jnp.float32
            )
            i += 1
    scale = np.asarray(sum_mask, dtype=np.float32) / np.float32(i)
    _cache["scale"] = scale
    return scale


def _build_nc():
    if "nc" in _cache:
        return _cache["nc"]
    import concourse.bass as bass
    import concourse.mybir as mybir
    from contextlib import ExitStack

    nc = bass.Bass(trn_type="TRN2")
    x = nc.dram_tensor(
        "x", [_ROWS_PER_CORE, _H], mybir.dt.float32, kind="ExternalInput"
    )
    scale = nc.dram_tensor(
        "scale", [128, _H], mybir.dt.float32, kind="ExternalInput"
    )
    y = nc.dram_tensor(
        "y", [_ROWS_PER_CORE, _H], mybir.dt.float32, kind="ExternalOutput"
    )

    # tile i = row-block i//2, column half i%2 — [128, 2048] each
    xv = x[:, :].rearrange("(n p) (c w) -> n c p w", p=128, w=_COLS_PER_TILE)
    yv = y[:, :].rearrange("(n p) (c w) -> n c p w", p=128, w=_COLS_PER_TILE)

    with ExitStack() as ctx:
        scale_sb = ctx.enter_context(
            nc.sbuf_tensor("scale_sb", [128, _H], mybir.dt.float32)
        )
        slots = [
            ctx.enter_context(
                nc.sbuf_tensor(
                    f"slot{s}", [128, _COLS_PER_TILE], mybir.dt.float32
                )
            )
            for s in range(_N_SLOTS)
        ]
        # One semaphore per slot: each slot has at most one outstanding DMA
        # at a time (load +16, store +16 → +32 per slot cycle), making the
        # wait thresholds exact.  A single shared DMA sem would race: the 16
        # SDMA engines increment independently per transfer, so "sem >=
        # 16*(i+1)" does not imply transfers 0..i all completed.
        slot_sems = [
            ctx.enter_context(nc.semaphore(f"slot_sem{s}"))
            for s in range(_N_SLOTS)
        ]
        mul_sem = ctx.enter_context(nc.semaphore("mul_sem"))
        sc_sem = ctx.enter_context(nc.semaphore("sc_sem"))
        block = ctx.enter_context(nc.Block())

        n_cycles = _N_TILES // _N_SLOTS

        # The i%2 column-half of the scale row that tile i multiplies by.
        def scale_ap(i):
            c = i % _TILES_PER_ROWBLK
            return scale_sb[:, c * _COLS_PER_TILE : (c + 1) * _COLS_PER_TILE]

        @block.sync
        def _(sync):
            # scale first on the same HWDGE queue: FIFO per SDMA engine
            # guarantees it lands before L0 without costing overlap
            sync.dma_start(out=scale_sb[:, :], in_=scale[:, :]).then_inc(
                sc_sem, 16
            )
            for i in range(_N_TILES):
                s, cyc = i % _N_SLOTS, i // _N_SLOTS
                if cyc > 0:
                    sync.wait_ge(slot_sems[s], 32 * cyc)  # prev store landed
                sync.dma_start(
                    out=slots[s][:, :],
                    in_=xv[i // _TILES_PER_ROWBLK, i % _TILES_PER_ROWBLK],
                ).then_inc(
                    slot_sems[s], 16
                )

        @block.vector
        def _(vector):
            vector.wait_ge(sc_sem, 16)
            for i in range(_N_TILES):
                s, cyc = i % _N_SLOTS, i // _N_SLOTS
                vector.wait_ge(slot_sems[s], 32 * cyc + 16)  # this load landed
                t = slots[s]
                vector.tensor_mul(
                    out=t[:, :], in0=t[:, :], in1=scale_ap(i)
                ).then_inc(mul_sem, 1)

        @block.scalar
        def _(scalar):
            for i in range(_N_TILES):
                s = i % _N_SLOTS
                scalar.wait_ge(mul_sem, i + 1)
                scalar.dma_start(
                    out=yv[i // _TILES_PER_ROWBLK, i % _TILES_PER_ROWBLK],
                    in_=slots[s][:, :],
                ).then_inc(
                    slot_sems[s], 16
                )
            # all output bytes landed before the program ends
            for s in range(_N_SLOTS):
                scalar.wait_ge(slot_sems[s], 32 * n_cycles)

    _cache["nc"] = nc
    return nc


def _run(x: np.ndarray, trace: bool = False, trace_cores=None):
    """Returns (full_output, BassKernelResults)."""
    from concourse.bass_utils import run_bass_kernel_spmd

    nc = _build_nc()
    scale_bc = np.ascontiguousarray(
        np.broadcast_to(_compute_scale()[None, :], (128, _H))
    )
    xf = np.ascontiguousarray(x, dtype=np.float32).reshape(_ROWS, _H)
    in_maps = [
        {"x": xf[c * _ROWS_PER_CORE : (c + 1) * _ROWS_PER_CORE], "scale": scale_bc}
        for c in range(_N_CORES)
    ]
    res = run_bass_kernel_spmd(
        nc,
        in_maps,
        core_ids=list(range(_N_CORES)),
        trace=trace,
        trace_cores=trace_cores,
    )
    out = np.concatenate([r["y"] for r in res.results], axis=0)
    return out.reshape(_B, _S, _H), res


def kernel(**inputs) -> np.ndarray:
    out, _ = _run(np.asarray(inputs["x"]))
    return out
